# revision 20
# baseline (speedup 1.0000x reference)
"""Multi-head attention (B=2, S=2048, E=1024, H=16) on 8 Trainium2 NeuronCores.

Sharding: tensor-parallel over heads — core i owns heads (2i, 2i+1), head A
on SBUF partitions 0:64, head B on 64:128 throughout.

  Phase A  (per core, per batch): q/k/v projections, feature-major. The
            k-projections for all token chunks are emitted first so the
            first score matmuls can start ~12us in. v is PE-transposed to
            token-major with a ones column per head (softmax-denominator
            trick, M=65 AV matmuls).
  Phase B/C (per core, per batch, per 256-query tile): scores as ROW-TILED
            matmul pairs — head A on PE rows 0:63 (tile_position (0,0)),
            head B on rows 64:127 ((64,0)), K=64 each, running concurrently
            in the array; this halves score-matmul issue time vs the
            zero-padded K=128 formulation. exp on ScalarE straight out of
            PSUM (no max-subtraction — scores are O(1) here); AV matmuls
            with the ones-row so the denominator falls out of the same fp32
            accumulation into a single-bank [65,2,256] PSUM tile.
            Normalize chain (denominator copy -> reciprocal_approx_fast ->
            partition broadcast -> multiply) runs off the PE critical path;
            double-buffered AV PSUM keeps the PE from ever waiting on it.
  AllToAll: one bf16 collective PER BATCH re-sharding head-parallel
            [128 feat, batch tokens] to token-parallel [all 1024 feat,
            256 tokens]; batch-0's collective and output projection overlap
            batch-1's attention compute.
  Phase D  (per core, per batch): output projection for a 256-token slice;
            phase D of batch 0 is emitted woven into batch-1's BC loop so
            its DMAs don't head-of-line-block the sync queue.

Matmuls run in bf16 (inputs cast on the host); PSUM accumulation is fp32.
"""

import numpy as np
import ml_dtypes

import concourse.bass as bass
import concourse.mybir as mybir
import concourse.tile as tile
from concourse import bacc
from concourse import bass_utils
from concourse.masks import make_identity

F32 = mybir.dt.float32
BF16 = mybir.dt.bfloat16
N_CORES = 8
P = 128

# Full problem dims (hardcoded per the harness contract)
B_FULL, S_FULL, E, H, D = 2, 2048, 1024, 16, 64
HPC = H // N_CORES            # heads per core = 2
F = HPC * D                   # feature cols per core = 128
SCALE = D ** -0.5


def build_nc(B=B_FULL, S=S_FULL):
    CDT = BF16
    T = B * S                 # tokens
    KO = E // P               # 8 contraction chunks over embed
    TC = min(512, S)          # phase-A token chunk
    NTC = S // TC             # chunks per batch
    Q2 = min(256, S)          # q tile
    NQ = S // Q2              # q tiles per batch (= N_CORES for S=2048)
    KC = S // P               # k chunks per batch
    G4 = min(2, KC)           # kc group per exp call
    QB = S // N_CORES         # tokens per dest core per batch (= Q2)
    TT = min(P, QB)           # phase-D token tile
    NT4 = QB // TT            # phase-D token tiles per batch

    nc = bacc.Bacc("TRN2", target_bir_lowering=False, debug=False,
                   num_devices=N_CORES)

    xT = nc.dram_tensor("xT", [E, T], CDT, kind="ExternalInput").ap()
    wq = nc.dram_tensor("wq", [E, F], CDT, kind="ExternalInput").ap()
    wk = nc.dram_tensor("wk", [E, F], CDT, kind="ExternalInput").ap()
    wv = nc.dram_tensor("wv", [E, F], CDT, kind="ExternalInput").ap()
    bq = nc.dram_tensor("bq", [F, 1], F32, kind="ExternalInput").ap()
    bk = nc.dram_tensor("bk", [F, 1], F32, kind="ExternalInput").ap()
    bv = nc.dram_tensor("bv", [F, 1], F32, kind="ExternalInput").ap()
    ow = nc.dram_tensor("ow", [E, E], CDT, kind="ExternalInput").ap()
    ob = nc.dram_tensor("ob", [1, E], F32, kind="ExternalInput").ap()
    # rows [b*QB + i]: batch b token core*QB + i
    out = nc.dram_tensor("out", [B * QB, E], F32, kind="ExternalOutput").ap()

    Exp = mybir.ActivationFunctionType.Exp

    with tile.TileContext(nc) as tc:
        with tc.tile_pool(name="persist", bufs=1) as persist, \
             tc.tile_pool(name="pA", bufs=4) as pA, \
             tc.tile_pool(name="pAv", bufs=2) as pAv, \
             tc.tile_pool(name="pBC", bufs=2) as pBC, \
             tc.tile_pool(name="pNr", bufs=3) as pNr, \
             tc.tile_pool(name="pGa", bufs=2) as pGa, \
             tc.tile_pool(name="pDo", bufs=2) as pDo, \
             tc.tile_pool(name="psA", bufs=2, space="PSUM") as psA, \
             tc.tile_pool(name="psS", bufs=2, space="PSUM") as psS, \
             tc.tile_pool(name="psAV", bufs=2, space="PSUM") as psAV, \
             tc.tile_pool(name="dramp", bufs=1, space="DRAM") as dramp:
            # k-weights + k-bias first on the DMA queue: the K projections
            # are the head of the critical path
            wk_sb = persist.tile([P, KO, F], CDT)
            wq_sb = persist.tile([P, KO, F], CDT)
            wv_sb = persist.tile([P, KO, F], CDT)
            bq_sb = persist.tile([P, 1], F32)
            bk_sb = persist.tile([P, 1], F32)
            bv_sb = persist.tile([P, 1], F32)
            nc.sync.dma_start(wk_sb, wk.rearrange("(ko p) f -> p ko f", p=P))
            nc.sync.dma_start(bk_sb, bk)

            xTr = xT.rearrange("(ko p) t -> p ko t", p=P)

            ident = persist.tile([P, P], CDT)
            make_identity(nc, ident)
            # dummy matmuls while the first input DMAs run: un-throttle the
            # PE (HAM) so the K projections issue at full clock
            ps_warm = psA.tile([P, P], F32, tag="psa", name="ps_warm")
            for _ in range(30):
                nc.tensor.matmul(ps_warm, lhsT=ident, rhs=ident,
                                 start=True, stop=True)

            # preload the exp table set while phase A runs
            warm = persist.tile([1, 1], F32)
            nc.vector.memset(warm, 0.0)
            nc.scalar.activation(warm, warm, Exp)

            qfm = persist.tile([P, T], CDT)     # q^T; head A rows 0:64, B 64:128
            kfm = persist.tile([P, T], CDT)     # k^T; same head layout
            # v token-major per 128-token chunk, with a ones column per head:
            # cols 0:64 head A v, col 64 ones, 65:129 head B v, col 129 ones
            vtm = persist.tile([P, T // P, 130], CDT)
            ones1 = persist.tile([P, 1], F32)
            nc.vector.memset(ones1, 1.0)
            nc.vector.tensor_copy(vtm[:, :, 64], ones1.to_broadcast([P, T // P]))
            nc.vector.tensor_copy(vtm[:, :, 129], ones1.to_broadcast([P, T // P]))

            a2a_in = [dramp.tile([N_CORES, P, QB], CDT, name=f"a2a_in{b}")
                      for b in range(B)]
            a2a_out = [dramp.tile([N_CORES, P, QB], CDT, name=f"a2a_out{b}")
                       for b in range(B)]

            ow_sb = persist.tile([P, KO, E], CDT)
            ob_row = persist.tile([1, E], F32)
            obb = persist.tile([P, E], F32)

            def phase_a(b):
                # K projections for every chunk first: scores can start
                # as soon as all-k plus the first q chunk are done.
                xts = []
                for tcx in range(NTC):
                    t0 = b * S + tcx * TC
                    xt = pA.tile([P, KO, TC], CDT, tag="xt")
                    nc.sync.dma_start(xt, xTr[:, :, t0:t0 + TC])
                    xts.append(xt)
                    ps = psA.tile([P, TC], F32, tag="psa")
                    for ko in range(KO):
                        nc.tensor.matmul(ps, lhsT=wk_sb[:, ko], rhs=xt[:, ko],
                                         start=(ko == 0), stop=(ko == KO - 1))
                    nc.vector.tensor_scalar_add(kfm[:, t0:t0 + TC], ps, bk_sb)
                if b == 0:
                    nc.sync.dma_start(
                        wq_sb, wq.rearrange("(ko p) f -> p ko f", p=P))
                    nc.sync.dma_start(
                        wv_sb, wv.rearrange("(ko p) f -> p ko f", p=P))
                    nc.sync.dma_start(bq_sb, bq)
                    nc.sync.dma_start(bv_sb, bv)
                for tcx in range(NTC):
                    t0 = b * S + tcx * TC
                    xt = xts[tcx]
                    ps = psA.tile([P, TC], F32, tag="psa")
                    for ko in range(KO):
                        nc.tensor.matmul(ps, lhsT=wq_sb[:, ko], rhs=xt[:, ko],
                                         start=(ko == 0), stop=(ko == KO - 1))
                    nc.vector.tensor_scalar_add(qfm[:, t0:t0 + TC], ps, bq_sb)
                    ps = psA.tile([P, TC], F32, tag="psa")
                    for ko in range(KO):
                        nc.tensor.matmul(ps, lhsT=wv_sb[:, ko], rhs=xt[:, ko],
                                         start=(ko == 0), stop=(ko == KO - 1))
                    vfm = pAv.tile([P, TC], CDT, tag="vfm")
                    nc.vector.tensor_scalar_add(vfm, ps, bv_sb)
                    for sub in range(TC // P):
                        pst = psA.tile([P, P], CDT, tag="psa")
                        nc.tensor.transpose(pst, vfm[:, sub * P:(sub + 1) * P],
                                            ident)
                        c = (t0 + sub * P) // P
                        nc.vector.tensor_copy(vtm[:, c, 0:64], pst[:, 0:64])
                        nc.vector.tensor_copy(vtm[:, c, 65:129], pst[:, 64:128])

            def phase_bc(b, hooks=None):
                for qi in range(NQ):
                    q0 = b * S + qi * Q2
                    eAB = pBC.tile([P, 2, KC, Q2], CDT, tag="exp")
                    for kg in range(KC // G4):
                        # one 4-bank PSUM tile per group: head A in the low
                        # banks, head B in the high banks. The single merged
                        # exp call keeps the A/B score matmuls' dependencies
                        # symmetric, so the scheduler leaves the row-tiled
                        # pairs adjacent and they run concurrently in the PE.
                        sP = psS.tile([P, 2, G4, Q2], F32, tag="sS")
                        for j in range(G4):
                            kc = kg * G4 + j
                            k0 = b * S + kc * P
                            # row-tiled pair: head A rows 0:63, head B 64:127
                            nc.tensor.matmul(
                                sP[:, 0, j], lhsT=kfm[0:64, k0:k0 + P],
                                rhs=qfm[0:64, q0:q0 + Q2],
                                start=True, stop=True)
                            nc.tensor.matmul(
                                sP[:, 1, j], lhsT=kfm[64:128, k0:k0 + P],
                                rhs=qfm[64:128, q0:q0 + Q2],
                                start=True, stop=True)
                        g0 = kg * G4
                        nc.scalar.activation(eAB[:, :, g0:g0 + G4, :], sP,
                                             Exp, scale=SCALE)
                    # numerators rows 0:64, denominator row 64; A/B side by
                    # side in a single PSUM bank so two q-tiles stay in flight
                    # one accumulation group across both heads: start=True
                    # resets has_written for the WHOLE bank, so only the very
                    # first matmul may carry it
                    pv = psAV.tile([65, 2, Q2], F32, tag="av")
                    for kc in range(KC):
                        c = (b * S) // P + kc
                        nc.tensor.matmul(pv[:, 0], lhsT=vtm[:, c, 0:65],
                                         rhs=eAB[:, 0, kc],
                                         start=(kc == 0), stop=False,
                                         skip_group_check=True)
                        nc.tensor.matmul(pv[:, 1], lhsT=vtm[:, c, 65:130],
                                         rhs=eAB[:, 1, kc],
                                         start=False, stop=(kc == KC - 1),
                                         skip_group_check=True)
                    # normalize chain — off the PE critical path
                    dsb = pNr.tile([P, 2, Q2], F32, tag="dsb")
                    nc.vector.tensor_copy(dsb[64:65], pv[64:65])
                    den = pNr.tile([1, 2, Q2], F32, tag="den")
                    nc.sync.dma_start(den, dsb[64:65])   # partition 64 -> 0
                    nc.vector.reciprocal_approx_fast(den, den)
                    db = pNr.tile([64, 2, Q2], F32, tag="db")
                    nc.gpsimd.partition_broadcast(db, den)
                    stage = pNr.tile([64, 2, Q2], CDT, tag="stage")
                    nc.vector.tensor_mul(stage[:, 0], pv[0:64, 0], db[:, 0])
                    nc.vector.tensor_mul(stage[:, 1], pv[0:64, 1], db[:, 1])
                    nc.sync.dma_start(
                        a2a_in[b][qi].rearrange("(h p) t -> p h t", h=HPC),
                        stage)
                    if hooks and qi in hooks:
                        hooks[qi]()

            def send_a2a(b):
                nc.gpsimd.collective_compute(
                    "AllToAll", mybir.AluOpType.bypass,
                    replica_groups=[list(range(N_CORES))],
                    ins=[a2a_in[b].opt()], outs=[a2a_out[b].opt()])

            def phase_d_pieces(b, wait_ms=None):
                """Returns [load_t4_0, load_t4_1, compute_t4_0, compute_t4_1].

                wait_ms: virtual-time floor for the Tile scheduler. Without
                it the scheduler hoists these a2a-dependent instructions to
                the engine queue heads (it models collectives as fast), and
                they head-of-line-block everything behind them for the real
                ~30-40us collective latency.
                """
                ga = pGa.tile([P, N_CORES, QB], CDT, tag="ga")
                a2a_out_r = a2a_out[b].rearrange("c p t -> p c t")
                pieces = []

                def load(t4):
                    def go():
                        from contextlib import nullcontext
                        cm = (tc.tile_wait_until(wait_ms)
                              if wait_ms is not None else nullcontext())
                        with cm:
                            nc.sync.dma_start(
                                ga[:, :, t4 * TT:(t4 + 1) * TT],
                                a2a_out_r[:, :, t4 * TT:(t4 + 1) * TT])
                    return go

                def compute(t4):
                    def go():
                        from contextlib import nullcontext
                        cm = (tc.tile_wait_until(wait_ms + 0.01)
                              if wait_ms is not None else nullcontext())
                        with cm:
                            for n2 in range(E // 512):
                                pso = psA.tile([P, 512], F32, tag="psa")
                                for r in range(N_CORES):
                                    nc.tensor.matmul(
                                        pso[0:TT],
                                        lhsT=ga[:, r, t4 * TT:(t4 + 1) * TT],
                                        rhs=ow_sb[:, r,
                                                  n2 * 512:(n2 + 1) * 512],
                                        start=(r == 0),
                                        stop=(r == N_CORES - 1))
                                osb = pDo.tile([TT, 512], F32, tag="osb")
                                nc.vector.tensor_add(
                                    osb, pso[0:TT],
                                    obb[0:TT, n2 * 512:(n2 + 1) * 512])
                                nc.sync.dma_start(
                                    out[b * QB + t4 * TT:
                                        b * QB + (t4 + 1) * TT,
                                        n2 * 512:(n2 + 1) * 512],
                                    osb)
                    return go

                for t4 in range(NT4):
                    pieces.append(load(t4))
                for t4 in range(NT4):
                    pieces.append(compute(t4))
                return pieces

            phase_a(0)
            phase_bc(0)
            send_a2a(0)
            phase_a(1)
            nc.sync.dma_start(ow_sb, ow.rearrange("(r p) e -> p r e", p=P))
            nc.sync.dma_start(ob_row, ob)
            nc.gpsimd.partition_broadcast(obb, ob_row)
            # weave phase D of batch 0 into batch 1's BC loop so its DMAs
            # queue behind already-satisfiable deps (a2a 0 done by then)
            d0 = phase_d_pieces(0, wait_ms=0.20)
            hooks = {4: d0[0], 5: d0[1], 6: d0[2], 7: d0[3]}
            phase_bc(1, hooks=hooks)
            send_a2a(1)
            # the PE idles through the final collective and HAM re-throttles
            # it to half clock; dummy matmuls pinned past the end of BC(1)
            # keep it warm so phase D of batch 1 issues at full rate
            with tc.tile_wait_until(0.225):
                ps_warm2 = psA.tile([P, P], F32, tag="psa", name="ps_warm2")
                for _ in range(60):
                    nc.tensor.matmul(ps_warm2, lhsT=ident, rhs=ident,
                                     start=True, stop=True)
            for piece in phase_d_pieces(1):
                piece()

    nc.compile()
    return nc


def make_in_maps(x, qkv_w, qkv_b, o_w, o_b, B=B_FULL, S=S_FULL):
    """Host-side sharding: full inputs -> per-core input dicts."""
    T = B * S
    idt = ml_dtypes.bfloat16
    x = np.asarray(x, dtype=np.float32)
    qkv_w = np.asarray(qkv_w, dtype=np.float32).astype(idt)
    qkv_b = np.asarray(qkv_b, dtype=np.float32)
    o_w = np.ascontiguousarray(np.asarray(o_w, dtype=np.float32).astype(idt))
    o_b = np.asarray(o_b, dtype=np.float32).reshape(1, E)
    xT = np.ascontiguousarray(x.reshape(T, E).T.astype(idt))
    in_maps = []
    for i in range(N_CORES):
        c0 = i * F
        in_maps.append({
            "xT": xT,
            "wq": np.ascontiguousarray(qkv_w[:, c0:c0 + F]),
            "wk": np.ascontiguousarray(qkv_w[:, E + c0:E + c0 + F]),
            "wv": np.ascontiguousarray(qkv_w[:, 2 * E + c0:2 * E + c0 + F]),
            "bq": np.ascontiguousarray(qkv_b[c0:c0 + F].reshape(F, 1)),
            "bk": np.ascontiguousarray(qkv_b[E + c0:E + c0 + F].reshape(F, 1)),
            "bv": np.ascontiguousarray(
                qkv_b[2 * E + c0:2 * E + c0 + F].reshape(F, 1)),
            "ow": o_w,
            "ob": o_b,
        })
    return in_maps


def gather_out(results, B=B_FULL, S=S_FULL):
    """Per-core [B*QB, E] slices -> full [B, S, E]."""
    QB = S // N_CORES
    full = np.empty((B, S, E), dtype=np.float32)
    for c in range(N_CORES):
        r = results[c]["out"]
        for b in range(B):
            full[b, c * QB:(c + 1) * QB] = r[b * QB:(b + 1) * QB]
    return full


_NC_CACHE = {}


def _get_nc(B=B_FULL, S=S_FULL):
    key = (B, S)
    if key not in _NC_CACHE:
        _NC_CACHE[key] = build_nc(B, S)
    return _NC_CACHE[key]


def kernel(x, qkv_w, qkv_b, o_w, o_b):
    B, S, _ = np.asarray(x).shape
    nc = _get_nc(B, S)
    in_maps = make_in_maps(x, qkv_w, qkv_b, o_w, o_b, B, S)
    res = bass_utils.run_bass_kernel_spmd(
        nc, in_maps, core_ids=list(range(N_CORES)))
    return gather_out(res.results, B, S)


# revision 21
# speedup vs baseline: 1.0060x; 1.0060x over previous
"""Multi-head attention (B=2, S=2048, E=1024, H=16) on 8 Trainium2 NeuronCores.

Sharding: tensor-parallel over heads — core i owns heads (2i, 2i+1), head A
on SBUF partitions 0:64, head B on 64:128 throughout.

  Phase A  (per core, per batch): q/k/v projections, feature-major. The
            k-projections for all token chunks are emitted first so the
            first score matmuls can start ~12us in. v is PE-transposed to
            token-major with a ones column per head (softmax-denominator
            trick, M=65 AV matmuls).
  Phase B/C (per core, per batch, per 256-query tile): scores as ROW-TILED
            matmul pairs — head A on PE rows 0:63 (tile_position (0,0)),
            head B on rows 64:127 ((64,0)), K=64 each, running concurrently
            in the array; this halves score-matmul issue time vs the
            zero-padded K=128 formulation. exp on ScalarE straight out of
            PSUM (no max-subtraction — scores are O(1) here); AV matmuls
            with the ones-row so the denominator falls out of the same fp32
            accumulation into a single-bank [65,2,256] PSUM tile.
            Normalize chain (denominator copy -> reciprocal_approx_fast ->
            partition broadcast -> multiply) runs off the PE critical path;
            double-buffered AV PSUM keeps the PE from ever waiting on it.
  AllToAll: one bf16 collective PER BATCH re-sharding head-parallel
            [128 feat, batch tokens] to token-parallel [all 1024 feat,
            256 tokens]; batch-0's collective and output projection overlap
            batch-1's attention compute.
  Phase D  (per core, per batch): output projection for a 256-token slice;
            phase D of batch 0 is emitted woven into batch-1's BC loop so
            its DMAs don't head-of-line-block the sync queue.

Matmuls run in bf16 (inputs cast on the host); PSUM accumulation is fp32.
"""

import numpy as np
import ml_dtypes

import concourse.bass as bass
import concourse.mybir as mybir
import concourse.tile as tile
from concourse import bacc
from concourse import bass_utils
from concourse.masks import make_identity

F32 = mybir.dt.float32
BF16 = mybir.dt.bfloat16
N_CORES = 8
P = 128

# Full problem dims (hardcoded per the harness contract)
B_FULL, S_FULL, E, H, D = 2, 2048, 1024, 16, 64
HPC = H // N_CORES            # heads per core = 2
F = HPC * D                   # feature cols per core = 128
SCALE = D ** -0.5


def build_nc(B=B_FULL, S=S_FULL):
    CDT = BF16
    T = B * S                 # tokens
    KO = E // P               # 8 contraction chunks over embed
    TC = min(512, S)          # phase-A token chunk
    NTC = S // TC             # chunks per batch
    Q2 = min(256, S)          # q tile
    NQ = S // Q2              # q tiles per batch (= N_CORES for S=2048)
    KC = S // P               # k chunks per batch
    G4 = min(2, KC)           # kc group per exp call
    QB = S // N_CORES         # tokens per dest core per batch (= Q2)
    TT = min(P, QB)           # phase-D token tile
    NT4 = QB // TT            # phase-D token tiles per batch

    nc = bacc.Bacc("TRN2", target_bir_lowering=False, debug=False,
                   num_devices=N_CORES)

    xT = nc.dram_tensor("xT", [E, T], CDT, kind="ExternalInput").ap()
    wq = nc.dram_tensor("wq", [E, F], CDT, kind="ExternalInput").ap()
    wk = nc.dram_tensor("wk", [E, F], CDT, kind="ExternalInput").ap()
    wv = nc.dram_tensor("wv", [E, F], CDT, kind="ExternalInput").ap()
    bq = nc.dram_tensor("bq", [F, 1], F32, kind="ExternalInput").ap()
    bk = nc.dram_tensor("bk", [F, 1], F32, kind="ExternalInput").ap()
    bv = nc.dram_tensor("bv", [F, 1], F32, kind="ExternalInput").ap()
    ow = nc.dram_tensor("ow", [E, E], CDT, kind="ExternalInput").ap()
    ob = nc.dram_tensor("ob", [1, E], F32, kind="ExternalInput").ap()
    # rows [b*QB + i]: batch b token core*QB + i
    out = nc.dram_tensor("out", [B * QB, E], F32, kind="ExternalOutput").ap()

    Exp = mybir.ActivationFunctionType.Exp

    with tile.TileContext(nc) as tc:
        with tc.tile_pool(name="persist", bufs=1) as persist, \
             tc.tile_pool(name="pA", bufs=4) as pA, \
             tc.tile_pool(name="pAv", bufs=2) as pAv, \
             tc.tile_pool(name="pBC", bufs=2) as pBC, \
             tc.tile_pool(name="pNr", bufs=3) as pNr, \
             tc.tile_pool(name="pGa", bufs=2) as pGa, \
             tc.tile_pool(name="pDo", bufs=2) as pDo, \
             tc.tile_pool(name="psA", bufs=2, space="PSUM") as psA, \
             tc.tile_pool(name="psS", bufs=2, space="PSUM") as psS, \
             tc.tile_pool(name="psAV", bufs=2, space="PSUM") as psAV, \
             tc.tile_pool(name="dramp", bufs=1, space="DRAM") as dramp:
            # k-weights + k-bias first on the DMA queue: the K projections
            # are the head of the critical path
            wk_sb = persist.tile([P, KO, F], CDT)
            wq_sb = persist.tile([P, KO, F], CDT)
            wv_sb = persist.tile([P, KO, F], CDT)
            bq_sb = persist.tile([P, 1], F32)
            bk_sb = persist.tile([P, 1], F32)
            bv_sb = persist.tile([P, 1], F32)
            nc.sync.dma_start(wk_sb, wk.rearrange("(ko p) f -> p ko f", p=P))
            nc.sync.dma_start(bk_sb, bk)

            xTr = xT.rearrange("(ko p) t -> p ko t", p=P)

            ident = persist.tile([P, P], CDT)
            make_identity(nc, ident)
            # dummy matmuls while the first input DMAs run: un-throttle the
            # PE (HAM) so the K projections issue at full clock
            ps_warm = psA.tile([P, P], F32, tag="psa", name="ps_warm")
            for _ in range(30):
                nc.tensor.matmul(ps_warm, lhsT=ident, rhs=ident,
                                 start=True, stop=True)

            # preload the exp table set while phase A runs
            warm = persist.tile([1, 1], F32)
            nc.vector.memset(warm, 0.0)
            nc.scalar.activation(warm, warm, Exp)

            qfm = persist.tile([P, T], CDT)     # q^T; head A rows 0:64, B 64:128
            kfm = persist.tile([P, T], CDT)     # k^T; same head layout
            # v token-major per 128-token chunk, with a ones column per head:
            # cols 0:64 head A v, col 64 ones, 65:129 head B v, col 129 ones
            vtm = persist.tile([P, T // P, 130], CDT)
            ones1 = persist.tile([P, 1], F32)
            nc.vector.memset(ones1, 1.0)
            nc.vector.tensor_copy(vtm[:, :, 64], ones1.to_broadcast([P, T // P]))
            nc.vector.tensor_copy(vtm[:, :, 129], ones1.to_broadcast([P, T // P]))

            a2a_in = [dramp.tile([N_CORES, P, QB], CDT, name=f"a2a_in{b}")
                      for b in range(B)]
            a2a_out = [dramp.tile([N_CORES, P, QB], CDT, name=f"a2a_out{b}")
                       for b in range(B)]

            ow_sb = persist.tile([P, KO, E], CDT)
            ob_row = persist.tile([1, E], F32)
            nc.sync.dma_start(ob_row, ob)
            obb = persist.tile([P, E], F32)
            nc.gpsimd.partition_broadcast(obb, ob_row)

            def phase_a(b):
                # K projections for every chunk first: scores can start
                # as soon as all-k plus the first q chunk are done.
                xts = []
                for tcx in range(NTC):
                    t0 = b * S + tcx * TC
                    xt = pA.tile([P, KO, TC], CDT, tag="xt")
                    nc.sync.dma_start(xt, xTr[:, :, t0:t0 + TC])
                    xts.append(xt)
                    ps = psA.tile([P, TC], F32, tag="psa")
                    for ko in range(KO):
                        nc.tensor.matmul(ps, lhsT=wk_sb[:, ko], rhs=xt[:, ko],
                                         start=(ko == 0), stop=(ko == KO - 1))
                    nc.vector.tensor_scalar_add(kfm[:, t0:t0 + TC], ps, bk_sb)
                if b == 0:
                    nc.sync.dma_start(
                        wq_sb, wq.rearrange("(ko p) f -> p ko f", p=P))
                    nc.sync.dma_start(
                        wv_sb, wv.rearrange("(ko p) f -> p ko f", p=P))
                    nc.sync.dma_start(bq_sb, bq)
                    nc.sync.dma_start(bv_sb, bv)
                for tcx in range(NTC):
                    t0 = b * S + tcx * TC
                    xt = xts[tcx]
                    ps = psA.tile([P, TC], F32, tag="psa")
                    for ko in range(KO):
                        nc.tensor.matmul(ps, lhsT=wq_sb[:, ko], rhs=xt[:, ko],
                                         start=(ko == 0), stop=(ko == KO - 1))
                    nc.vector.tensor_scalar_add(qfm[:, t0:t0 + TC], ps, bq_sb)
                    ps = psA.tile([P, TC], F32, tag="psa")
                    for ko in range(KO):
                        nc.tensor.matmul(ps, lhsT=wv_sb[:, ko], rhs=xt[:, ko],
                                         start=(ko == 0), stop=(ko == KO - 1))
                    vfm = pAv.tile([P, TC], CDT, tag="vfm")
                    nc.vector.tensor_scalar_add(vfm, ps, bv_sb)
                    for sub in range(TC // P):
                        pst = psA.tile([P, P], CDT, tag="psa")
                        nc.tensor.transpose(pst, vfm[:, sub * P:(sub + 1) * P],
                                            ident)
                        c = (t0 + sub * P) // P
                        nc.vector.tensor_copy(vtm[:, c, 0:64], pst[:, 0:64])
                        nc.vector.tensor_copy(vtm[:, c, 65:129], pst[:, 64:128])

            def phase_bc(b, hooks=None):
                for qi in range(NQ):
                    q0 = b * S + qi * Q2
                    eAB = pBC.tile([P, 2, KC, Q2], CDT, tag="exp")
                    for kg in range(KC // G4):
                        # one 4-bank PSUM tile per group: head A in the low
                        # banks, head B in the high banks. The single merged
                        # exp call keeps the A/B score matmuls' dependencies
                        # symmetric, so the scheduler leaves the row-tiled
                        # pairs adjacent and they run concurrently in the PE.
                        sP = psS.tile([P, 2, G4, Q2], F32, tag="sS")
                        for j in range(G4):
                            kc = kg * G4 + j
                            k0 = b * S + kc * P
                            # row-tiled pair: head A rows 0:63, head B 64:127
                            nc.tensor.matmul(
                                sP[:, 0, j], lhsT=kfm[0:64, k0:k0 + P],
                                rhs=qfm[0:64, q0:q0 + Q2],
                                start=True, stop=True)
                            nc.tensor.matmul(
                                sP[:, 1, j], lhsT=kfm[64:128, k0:k0 + P],
                                rhs=qfm[64:128, q0:q0 + Q2],
                                start=True, stop=True)
                        g0 = kg * G4
                        nc.scalar.activation(eAB[:, :, g0:g0 + G4, :], sP,
                                             Exp, scale=SCALE)
                    # numerators rows 0:64, denominator row 64; A/B side by
                    # side in a single PSUM bank so two q-tiles stay in flight
                    # one accumulation group across both heads: start=True
                    # resets has_written for the WHOLE bank, so only the very
                    # first matmul may carry it
                    pv = psAV.tile([65, 2, Q2], F32, tag="av")
                    for kc in range(KC):
                        c = (b * S) // P + kc
                        nc.tensor.matmul(pv[:, 0], lhsT=vtm[:, c, 0:65],
                                         rhs=eAB[:, 0, kc],
                                         start=(kc == 0), stop=False,
                                         skip_group_check=True)
                        nc.tensor.matmul(pv[:, 1], lhsT=vtm[:, c, 65:130],
                                         rhs=eAB[:, 1, kc],
                                         start=False, stop=(kc == KC - 1),
                                         skip_group_check=True)
                    # normalize chain — off the PE critical path
                    dsb = pNr.tile([P, 2, Q2], F32, tag="dsb")
                    nc.vector.tensor_copy(dsb[64:65], pv[64:65])
                    den = pNr.tile([1, 2, Q2], F32, tag="den")
                    nc.sync.dma_start(den, dsb[64:65])   # partition 64 -> 0
                    nc.vector.reciprocal_approx_fast(den, den)
                    db = pNr.tile([64, 2, Q2], F32, tag="db")
                    nc.gpsimd.partition_broadcast(db, den)
                    stage = pNr.tile([64, 2, Q2], CDT, tag="stage")
                    nc.vector.tensor_mul(stage[:, 0], pv[0:64, 0], db[:, 0])
                    nc.vector.tensor_mul(stage[:, 1], pv[0:64, 1], db[:, 1])
                    nc.sync.dma_start(
                        a2a_in[b][qi].rearrange("(h p) t -> p h t", h=HPC),
                        stage)
                    if hooks and qi in hooks:
                        hooks[qi]()

            def send_a2a(b):
                nc.gpsimd.collective_compute(
                    "AllToAll", mybir.AluOpType.bypass,
                    replica_groups=[list(range(N_CORES))],
                    ins=[a2a_in[b].opt()], outs=[a2a_out[b].opt()])

            def phase_d_pieces(b, wait_ms=None):
                """Returns [load_t4_0, load_t4_1, compute_t4_0, compute_t4_1].

                wait_ms: virtual-time floor for the Tile scheduler. Without
                it the scheduler hoists these a2a-dependent instructions to
                the engine queue heads (it models collectives as fast), and
                they head-of-line-block everything behind them for the real
                ~30-40us collective latency.
                """
                ga = pGa.tile([P, N_CORES, QB], CDT, tag="ga")
                a2a_out_r = a2a_out[b].rearrange("c p t -> p c t")
                pieces = []

                def load(t4):
                    def go():
                        from contextlib import nullcontext
                        cm = (tc.tile_wait_until(wait_ms)
                              if wait_ms is not None else nullcontext())
                        with cm:
                            nc.sync.dma_start(
                                ga[:, :, t4 * TT:(t4 + 1) * TT],
                                a2a_out_r[:, :, t4 * TT:(t4 + 1) * TT])
                    return go

                def compute(t4):
                    def go():
                        from contextlib import nullcontext
                        cm = (tc.tile_wait_until(wait_ms + 0.01)
                              if wait_ms is not None else nullcontext())
                        with cm:
                            for n2 in range(E // 512):
                                pso = psA.tile([P, 512], F32, tag="psa")
                                for r in range(N_CORES):
                                    nc.tensor.matmul(
                                        pso[0:TT],
                                        lhsT=ga[:, r, t4 * TT:(t4 + 1) * TT],
                                        rhs=ow_sb[:, r,
                                                  n2 * 512:(n2 + 1) * 512],
                                        start=(r == 0),
                                        stop=(r == N_CORES - 1))
                                osb = pDo.tile([TT, 512], F32, tag="osb")
                                nc.vector.tensor_add(
                                    osb, pso[0:TT],
                                    obb[0:TT, n2 * 512:(n2 + 1) * 512])
                                nc.sync.dma_start(
                                    out[b * QB + t4 * TT:
                                        b * QB + (t4 + 1) * TT,
                                        n2 * 512:(n2 + 1) * 512],
                                    osb)
                    return go

                for t4 in range(NT4):
                    pieces.append(load(t4))
                for t4 in range(NT4):
                    pieces.append(compute(t4))
                return pieces

            phase_a(0)
            phase_bc(0)
            send_a2a(0)
            phase_a(1)
            nc.sync.dma_start(ow_sb, ow.rearrange("(r p) e -> p r e", p=P))
            # weave phase D of batch 0 into batch 1's BC loop so its DMAs
            # queue behind already-satisfiable deps (a2a 0 done by then)
            d0 = phase_d_pieces(0, wait_ms=0.20)
            hooks = {4: d0[0], 5: d0[1], 6: d0[2], 7: d0[3]}
            phase_bc(1, hooks=hooks)
            send_a2a(1)
            for piece in phase_d_pieces(1):
                piece()

    nc.compile()
    return nc


def make_in_maps(x, qkv_w, qkv_b, o_w, o_b, B=B_FULL, S=S_FULL):
    """Host-side sharding: full inputs -> per-core input dicts."""
    T = B * S
    idt = ml_dtypes.bfloat16
    x = np.asarray(x, dtype=np.float32)
    qkv_w = np.asarray(qkv_w, dtype=np.float32).astype(idt)
    qkv_b = np.asarray(qkv_b, dtype=np.float32)
    o_w = np.ascontiguousarray(np.asarray(o_w, dtype=np.float32).astype(idt))
    o_b = np.asarray(o_b, dtype=np.float32).reshape(1, E)
    xT = np.ascontiguousarray(x.reshape(T, E).T.astype(idt))
    in_maps = []
    for i in range(N_CORES):
        c0 = i * F
        in_maps.append({
            "xT": xT,
            "wq": np.ascontiguousarray(qkv_w[:, c0:c0 + F]),
            "wk": np.ascontiguousarray(qkv_w[:, E + c0:E + c0 + F]),
            "wv": np.ascontiguousarray(qkv_w[:, 2 * E + c0:2 * E + c0 + F]),
            "bq": np.ascontiguousarray(qkv_b[c0:c0 + F].reshape(F, 1)),
            "bk": np.ascontiguousarray(qkv_b[E + c0:E + c0 + F].reshape(F, 1)),
            "bv": np.ascontiguousarray(
                qkv_b[2 * E + c0:2 * E + c0 + F].reshape(F, 1)),
            "ow": o_w,
            "ob": o_b,
        })
    return in_maps


def gather_out(results, B=B_FULL, S=S_FULL):
    """Per-core [B*QB, E] slices -> full [B, S, E]."""
    QB = S // N_CORES
    full = np.empty((B, S, E), dtype=np.float32)
    for c in range(N_CORES):
        r = results[c]["out"]
        for b in range(B):
            full[b, c * QB:(c + 1) * QB] = r[b * QB:(b + 1) * QB]
    return full


_NC_CACHE = {}


def _get_nc(B=B_FULL, S=S_FULL):
    key = (B, S)
    if key not in _NC_CACHE:
        _NC_CACHE[key] = build_nc(B, S)
    return _NC_CACHE[key]


def kernel(x, qkv_w, qkv_b, o_w, o_b):
    B, S, _ = np.asarray(x).shape
    nc = _get_nc(B, S)
    in_maps = make_in_maps(x, qkv_w, qkv_b, o_w, o_b, B, S)
    res = bass_utils.run_bass_kernel_spmd(
        nc, in_maps, core_ids=list(range(N_CORES)))
    return gather_out(res.results, B, S)
